# revision 1
# baseline (speedup 1.0000x reference)
import sys

sys.path.insert(0, "/opt/trn_rl_repo")

import numpy as np  # noqa: E402

import concourse.bass as bass  # noqa: E402
import concourse.mybir as mybir  # noqa: E402
import concourse.tile as tile  # noqa: E402
from contextlib import ExitStack  # noqa: E402
from concourse import bacc  # noqa: E402
from concourse.bass_utils import run_bass_kernel_spmd  # noqa: E402
from concourse.masks import make_identity  # noqa: E402

F32 = mybir.dt.float32
BF16 = mybir.dt.bfloat16
AF = mybir.ActivationFunctionType
ALU = mybir.AluOpType
AX = mybir.AxisListType

S = 4  # samples per core
C, H, W = 256, 28, 28
N = H * W  # 784
NK = 196
HEADS, DK = 8, 32
CM = 1024
SCALE = DK ** -0.5
EPS = 1e-5
INV_NTOT = 1.0 / (C * N)
ISL = [(0, 512), (512, 272)]  # bank-aligned free splits of 784
NCORES = 8

_CACHE = {}


def _build():
    if "nc" in _CACHE:
        return _CACHE["nc"]
    nc = bacc.Bacc()

    x_d = nc.dram_tensor("x", [S, C, H, W], F32, kind="ExternalInput")
    y_d = nc.dram_tensor("y", [S, C, H, W], F32, kind="ExternalOutput")
    scr_d = nc.dram_tensor("scr", [S, N * C], F32)

    def din(name, shape):
        return nc.dram_tensor(name, shape, F32, kind="ExternalInput")

    lpu_w = din("lpu_w", [C, 1, 3, 3]); lpu_b = din("lpu_b", [C])
    dw_w = din("dw_w", [C, 1, 2, 2]); dw_b = din("dw_b", [C])
    wq = din("wq", [C, C]); bq = din("bq", [C])
    wk = din("wk", [C, C]); bk = din("bk", [C])
    wv = din("wv", [C, C]); bv = din("bv", [C])
    wo = din("wo", [C, C]); bo = din("bo", [C])
    pos_b = din("pos_b", [1, HEADS, N, NK])
    c1_w = din("c1_w", [CM, C, 1, 1]); c1_b = din("c1_b", [CM])
    bn1_g = din("bn1_g", [CM]); bn1_b = din("bn1_b", [CM])
    bn1_m = din("bn1_m", [CM]); bn1_v = din("bn1_v", [CM])
    dw2_w = din("dw2_w", [CM, 1, 3, 3]); dw2_b = din("dw2_b", [CM])
    bn2_g = din("bn2_g", [CM]); bn2_b = din("bn2_b", [CM])
    bn2_m = din("bn2_m", [CM]); bn2_v = din("bn2_v", [CM])
    c2_w = din("c2_w", [C, CM, 1, 1]); c2_b = din("c2_b", [C])
    bn3_g = din("bn3_g", [C]); bn3_b = din("bn3_b", [C])
    bn3_m = din("bn3_m", [C]); bn3_v = din("bn3_v", [C])

    xv = x_d.rearrange("s c h w -> s c (h w)")
    yv = y_d.rearrange("s c h w -> s c (h w)")

    with tile.TileContext(nc) as tc, ExitStack() as stk:
        cst = stk.enter_context(tc.tile_pool(name="cst", bufs=1))
        wk2 = stk.enter_context(tc.tile_pool(name="wk2", bufs=2))
        wk1 = stk.enter_context(tc.tile_pool(name="wk1", bufs=1))
        psA = stk.enter_context(tc.tile_pool(name="psA", bufs=3, space="PSUM"))
        psS = stk.enter_context(tc.tile_pool(name="psS", bufs=2, space="PSUM"))
        bootcm = tc.tile_pool(name="boot", bufs=2)
        boot = bootcm.__enter__()

        def pat(shape=(128, N)):
            return psA.tile(list(shape), F32, tag="attn", name="pat")

        def psm(shape=(128, 392)):
            return psS.tile(list(shape), F32, tag="small", name="psm")

        # ---------- one-time constants ----------
        ident = cst.tile([128, 128], BF16, tag="ident")
        make_identity(nc, ident)
        ones1 = cst.tile([1, 128], BF16, tag="ones1")
        nc.vector.memset(ones1, 1.0)
        onesM = cst.tile([128, 128], F32, tag="onesM")
        nc.vector.memset(onesM, 1.0)
        ones392 = cst.tile([1, 392], BF16, tag="ones392")
        nc.vector.memset(ones392, 1.0)
        eps128 = cst.tile([128, 1], F32, tag="eps128")
        nc.vector.memset(eps128, EPS)

        bh = []
        for q in range(4):
            t = cst.tile([128, 128], BF16, tag=f"bh{q}")
            nc.vector.memset(t, 0.0)
            nc.vector.memset(t[:, 32 * q:32 * q + 32], 1.0)
            bh.append(t)

        # depthwise conv weights -> per-tap diagonal matrices (bf16)
        def conv_diags(w_dram, nch, ktaps, name):
            diags = []
            wflat = w_dram.rearrange("c one a b -> c (one a b)")
            for g in range(nch):
                wsb = boot.tile([128, ktaps], F32, tag="wsb")
                nc.sync.dma_start(out=wsb, in_=wflat[g * 128:(g + 1) * 128, :])
                row = []
                for t in range(ktaps):
                    dg = cst.tile([128, 128], BF16, tag=f"dg_{name}_{g}_{t}")
                    nc.vector.tensor_scalar(
                        out=dg, in0=ident, scalar1=wsb[:, t:t + 1], scalar2=None,
                        op0=ALU.mult)
                    row.append(dg)
                diags.append(row)
            return diags

        dg_lpu = conv_diags(lpu_w, 2, 9, "lpu")
        dg_kv = conv_diags(dw_w, 2, 4, "kv")
        dw2w = []
        dw2f = dw2_w.rearrange("c one a b -> c (one a b)")
        for g in range(8):
            t = cst.tile([128, 9], F32, tag=f"dw2w{g}")
            nc.sync.dma_start(out=t, in_=dw2f[g * 128:(g + 1) * 128, :])
            dw2w.append(t)

        # transposed bf16 weight tiles
        def load_wT(w_dram, km, mm, name):
            # w [M=mm*128, K=km*128] row-major -> list of km tiles [128, mm*128]
            out = [cst.tile([128, mm * 128], BF16, tag=f"wT_{name}_{k}",
                            name=f"wT_{name}_{k}")
                   for k in range(km)]
            for mc in range(mm):
                wtmp = boot.tile([128, km * 128], F32, tag="wtmp")
                nc.sync.dma_start(out=wtmp, in_=w_dram[mc * 128:(mc + 1) * 128, :])
                wtb = boot.tile([128, km * 128], BF16, tag="wtb")
                nc.vector.tensor_copy(out=wtb, in_=wtmp)
                for kc in range(km):
                    eng = nc.sync if kc % 2 == 0 else nc.scalar
                    eng.dma_start_transpose(
                        out=out[kc][:, mc * 128:(mc + 1) * 128],
                        in_=wtb[:, kc * 128:(kc + 1) * 128])
            return out

        wqT = load_wT(wq, 2, 2, "wq")
        wkT = load_wT(wk, 2, 2, "wk")
        wvT = load_wT(wv, 2, 2, "wv")
        woT = load_wT(wo, 2, 2, "wo")
        c1T = load_wT(c1_w.rearrange("m k one one2 -> m (k one one2)"), 2, 8, "c1")
        c2T = load_wT(c2_w.rearrange("m k one one2 -> m (k one one2)"), 8, 2, "c2")

        # BN affine folds: A = g*rsqrt(v+eps), B = b - m*A + A*conv_bias
        def bn_fold(g_d, b_d, m_d, v_d, cb_d, nch, name):
            A = cst.tile([128, nch], F32, tag=f"A_{name}")
            B = cst.tile([128, nch], F32, tag=f"B_{name}")
            gs = boot.tile([128, nch], F32, tag="gs")
            bs = boot.tile([128, nch], F32, tag="bs")
            ms = boot.tile([128, nch], F32, tag="ms")
            vs = boot.tile([128, nch], F32, tag="vs")
            cb = boot.tile([128, nch], F32, tag="cb")
            for t, d in ((gs, g_d), (bs, b_d), (ms, m_d), (vs, v_d), (cb, cb_d)):
                nc.sync.dma_start(out=t, in_=d.rearrange("(a p) -> p a", p=128))
            sq = boot.tile([128, nch], F32, tag="sqb")
            nc.scalar.activation(out=sq, in_=vs, func=AF.Ln, bias=eps128)
            rc = boot.tile([128, nch], F32, tag="rcb")
            nc.scalar.activation(out=rc, in_=sq, func=AF.Exp, scale=-0.5)
            nc.vector.tensor_mul(out=A, in0=gs, in1=rc)
            t1 = boot.tile([128, nch], F32, tag="t1b")
            nc.vector.tensor_mul(out=t1, in0=ms, in1=A)
            nc.vector.tensor_sub(out=B, in0=bs, in1=t1)
            nc.vector.tensor_mul(out=t1, in0=cb, in1=A)
            nc.vector.tensor_add(out=B, in0=B, in1=t1)
            return A, B

        A1, B1 = bn_fold(bn1_g, bn1_b, bn1_m, bn1_v, c1_b, 8, "bn1")
        A2, B2 = bn_fold(bn2_g, bn2_b, bn2_m, bn2_v, dw2_b, 8, "bn2")
        A3, B3 = bn_fold(bn3_g, bn3_b, bn3_m, bn3_v, c2_b, 2, "bn3")

        # bias columns [128,1]
        def bias_cols(d, nch, name):
            out = []
            for g in range(nch):
                t = cst.tile([128, 1], F32, tag=f"bc_{name}_{g}")
                nc.sync.dma_start(
                    out=t, in_=d[g * 128:(g + 1) * 128].rearrange("(c one) -> c one", one=1))
                out.append(t)
            return out

        lpub = bias_cols(lpu_b, 2, "lpub")
        lpubr = []
        for g in range(2):
            t = wk2.tile([1, 128], F32, tag="lpubrf")
            nc.sync.dma_start(
                out=t, in_=lpu_b[g * 128:(g + 1) * 128].rearrange(
                    "(one c) -> one c", one=1))
            tb = cst.tile([1, 128], BF16, tag=f"lpubr{g}")
            nc.vector.tensor_copy(out=tb, in_=t)
            lpubr.append(tb)
        dwb = bias_cols(dw_b, 2, "dwb")
        bqc = bias_cols(bq, 2, "bq")
        bkc = bias_cols(bk, 2, "bk")

        def bias_row(d, name):
            tf = boot.tile([1, C], F32, tag="brf")
            nc.sync.dma_start(out=tf, in_=d.rearrange("(one c) -> one c", one=1))
            t = cst.tile([1, C], BF16, tag=f"br_{name}")
            nc.vector.tensor_copy(out=t, in_=tf)
            return t

        bo_r = bias_row(bo, "bo")
        bv_r = bias_row(bv, "bv")

        # E = exp(pos_b)^T per head: [128,1568] bf16, cols 0:784 = j 0:127,
        # cols 784:1568 rows 0:68 = j 128:196
        Ec = []
        for h in range(HEADS):
            pt = boot.tile([128, 2 * N], BF16, tag="posT")
            for pc in range(7):
                pbp = boot.tile([128, 256], BF16, tag="pbp")
                nc.vector.memset(pbp[:, NK:], 0.0)
                nc.gpsimd.dma_start(
                    out=pbp[:112, :NK],
                    in_=pos_b[0, h, pc * 112:(pc + 1) * 112, :])
                eng = nc.sync if pc % 2 == 0 else nc.scalar
                eng.dma_start_transpose(
                    out=pt[:, pc * 112:(pc + 1) * 112], in_=pbp[:112, 0:128])
                eng.dma_start_transpose(
                    out=pt[:, N + pc * 112:N + (pc + 1) * 112], in_=pbp[:112, 128:256])
            e = cst.tile([128, 2 * N], BF16, tag=f"E{h}")
            nc.scalar.activation(out=e, in_=pt, func=AF.Exp)
            Ec.append(e)
        bootcm.__exit__(None, None, None)

        # LN over (C,H,W): returns (mean, rstd) as [1,1] APs
        def ln_stats(chunks, tagp):
            st4 = wk2.tile([128, 4], F32, tag=f"st4{tagp}")
            for ch in range(2):
                nc.vector.tensor_reduce(
                    out=st4[:, 2 * ch:2 * ch + 1], in_=chunks[ch], axis=AX.X,
                    op=ALU.add)
                scr = wk2.tile([128, N], BF16, tag="lnsc")
                nc.scalar.activation(
                    out=scr, in_=chunks[ch], func=AF.Square,
                    accum_out=st4[:, 2 * ch + 1:2 * ch + 2])
            pst = psm((128, 4))
            nc.tensor.matmul(pst, onesM, st4, start=True, stop=True)
            stc = wk2.tile([128, 4], F32, tag=f"stc{tagp}")
            nc.vector.tensor_copy(out=stc, in_=pst)
            tot = wk2.tile([128, 2], F32, tag=f"tot{tagp}")
            nc.vector.tensor_add(out=tot, in0=stc[:, 0:2], in1=stc[:, 2:4])
            mv = wk2.tile([128, 2], F32, tag=f"mv{tagp}")
            nc.vector.tensor_scalar(
                out=mv, in0=tot, scalar1=INV_NTOT, scalar2=None, op0=ALU.mult)
            m2 = wk2.tile([128, 1], F32, tag=f"m2{tagp}")
            nc.vector.tensor_mul(out=m2, in0=mv[:, 0:1], in1=mv[:, 0:1])
            var = wk2.tile([128, 1], F32, tag=f"var{tagp}")
            nc.vector.tensor_sub(out=var, in0=mv[:, 1:2], in1=m2)
            lg = wk2.tile([128, 1], F32, tag=f"lg{tagp}")
            nc.scalar.activation(out=lg, in_=var, func=AF.Ln, bias=eps128)
            rst = wk2.tile([128, 1], F32, tag=f"rst{tagp}")
            nc.scalar.activation(out=rst, in_=lg, func=AF.Exp, scale=-0.5)
            return mv[:, 0:1], rst

        def emit_ffn(s, x2, ln2b):
                h1p = []
                for mc in range(8):
                    pc1 = pat()
                    for i0, iw in ISL:
                        for kc in range(2):
                            nc.tensor.matmul(
                                pc1[:, i0:i0 + iw],
                                c1T[kc][:, mc * 128:(mc + 1) * 128],
                                ln2b[kc][:, i0:i0 + iw],
                                start=(kc == 0), stop=(kc == 1))
                    hp = wk1.tile([128, 30, 30], BF16, tag=f"h1p{mc}")
                    if s == 0:
                        nc.vector.memset(hp, 0.0)
                    nc.scalar.activation(
                        out=hp[:, 1:29, 1:29],
                        in_=pc1.rearrange("p (h w) -> p h w", w=W),
                        func=AF.Gelu, scale=A1[:, mc:mc + 1], bias=B1[:, mc:mc + 1])
                    h1p.append(hp)
                h2 = []
                for mc in range(8):
                    dgs = []
                    for t9 in range(9):
                        dg = wk2.tile([128, 128], BF16, tag="dgdw", bufs=18,
                                      name="dgdw")
                        nc.vector.tensor_scalar(
                            out=dg, in0=ident, scalar1=dw2w[mc][:, t9:t9 + 1],
                            scalar2=None, op0=ALU.mult)
                        dgs.append(dg)
                    t = wk1.tile([128, N], BF16, tag=f"h2{mc}")
                    for hf in range(2):
                        pd = psm()
                        for t9 in range(9):
                            dy, dx = t9 // 3, t9 % 3
                            nc.tensor.matmul(
                                pd, dgs[t9],
                                h1p[mc][:, dy + 14 * hf:dy + 14 * hf + 14, dx:dx + 28],
                                start=(t9 == 0), stop=(t9 == 8))
                        nc.scalar.activation(
                            out=t[:, hf * 392:(hf + 1) * 392], in_=pd,
                            func=AF.Gelu, scale=A2[:, mc:mc + 1], bias=B2[:, mc:mc + 1])
                    h2.append(t)
                for mc in range(2):
                    pc2 = pat()
                    for i0, iw in ISL:
                        for kc in range(8):
                            nc.tensor.matmul(
                                pc2[:, i0:i0 + iw],
                                c2T[kc][:, mc * 128:(mc + 1) * 128],
                                h2[kc][:, i0:i0 + iw],
                                start=(kc == 0), stop=(kc == 7))
                    t3 = wk2.tile([128, N], F32, tag="t3")
                    nc.vector.tensor_scalar(
                        out=t3, in0=pc2, scalar1=A3[:, mc:mc + 1],
                        scalar2=B3[:, mc:mc + 1], op0=ALU.mult, op1=ALU.add)
                    nc.vector.tensor_add(out=t3, in0=t3, in1=x2[mc])
                    nc.sync.dma_start(
                        out=yv[s, mc * 128:(mc + 1) * 128, :], in_=t3)


        # ================= per-sample pipeline =================
        samples = []
        for s in range(S):
            # ---- A: load + LPU ----
            xs, xb, x1, x1b = [], [], [], []
            for ch in range(2):
                t = wk2.tile([128, N], F32, tag="xs")
                nc.sync.dma_start(out=t, in_=xv[s, ch * 128:(ch + 1) * 128, :])
                xs.append(t)
                p = wk2.tile([128, 30, 30], BF16, tag=f"xb{ch}", bufs=1)
                if s == 0:
                    nc.vector.memset(p, 0.0)
                nc.vector.tensor_copy(
                    out=p[:, 1:29, 1:29],
                    in_=t.rearrange("p (h w) -> p h w", w=W))
                xb.append(p)
            for ch in range(2):
                t = wk2.tile([128, N], F32, tag=f"x1{ch}", bufs=1)
                for hf in range(2):
                    pl = psm()
                    for t9 in range(9):
                        dy, dx = t9 // 3, t9 % 3
                        nc.tensor.matmul(
                            pl, dg_lpu[ch][t9],
                            xb[ch][:, dy + 14 * hf:dy + 14 * hf + 14, dx:dx + 28],
                            start=(t9 == 0), stop=(t9 == 8))
                    nc.vector.tensor_scalar(
                        out=t[:, hf * 392:(hf + 1) * 392], in0=pl,
                        scalar1=lpub[ch], scalar2=None, op0=ALU.add)
                nc.vector.tensor_add(out=t, in0=t, in1=xs[ch])
                x1.append(t)
                tb = wk2.tile([128, N], BF16, tag=f"x1b{ch}")
                nc.vector.tensor_copy(out=tb, in_=t)
                x1b.append(tb)

            # ---- B: LN1 ----
            mean1, rstd1 = ln_stats(x1, "l1")
            ln1b = []
            for ch in range(2):
                t = wk2.tile([128, N], BF16, tag=f"ln1b{ch}")
                nc.vector.tensor_scalar(
                    out=t, in0=x1[ch], scalar1=mean1, scalar2=rstd1,
                    op0=ALU.subtract, op1=ALU.mult)
                ln1b.append(t)

            # ---- C: kv conv (2x2 stride 2 on x1) ----
            kvb = []
            for ch in range(2):
                x5 = x1b[ch].rearrange(
                    "p (h a w b) -> p h a w b", h=14, a=2, w=14, b=2)
                pk = psm((128, NK))
                for t4 in range(4):
                    dy, dx = t4 // 2, t4 % 2
                    nc.tensor.matmul(
                        pk, dg_kv[ch][t4], x5[:, :, dy, :, dx],
                        start=(t4 == 0), stop=(t4 == 3))
                t = wk2.tile([128, NK], BF16, tag=f"kvb{ch}")
                nc.scalar.activation(out=t, in_=pk, func=AF.Identity, bias=dwb[ch])
                kvb.append(t)

            # ---- D: q/k/v projections ----
            qb = []
            for mc in range(2):
                pq = pat()
                for i0, iw in ISL:
                    for kc in range(2):
                        nc.tensor.matmul(
                            pq[:, i0:i0 + iw],
                            wqT[kc][:, mc * 128:(mc + 1) * 128],
                            ln1b[kc][:, i0:i0 + iw],
                            start=(kc == 0), stop=(kc == 1))
                t = wk2.tile([128, N], BF16, tag=f"qb{mc}")
                nc.vector.tensor_scalar(
                    out=t, in0=pq, scalar1=bqc[mc], scalar2=SCALE,
                    op0=ALU.add, op1=ALU.mult)
                qb.append(t)
            kb = []
            for mc in range(2):
                pk2 = psm((128, NK))
                for kc in range(2):
                    nc.tensor.matmul(
                        pk2, wkT[kc][:, mc * 128:(mc + 1) * 128], kvb[kc],
                        start=(kc == 0), stop=(kc == 1))
                t = wk2.tile([128, NK], BF16, tag=f"kb{mc}")
                nc.vector.tensor_scalar(
                    out=t, in0=pk2, scalar1=bkc[mc], scalar2=None, op0=ALU.add)
                kb.append(t)
            vb = []
            for pi, (j0, jw) in enumerate([(0, 128), (128, 68)]):
                pv = psm((128, C))
                nc.tensor.matmul(
                    pv[0:jw, :], ones1[0:1, 0:jw], bv_r, start=True, stop=False)
                for kc in range(2):
                    nc.tensor.matmul(
                        pv[0:jw, :], kvb[kc][:, j0:j0 + jw], wvT[kc],
                        start=False, stop=(kc == 1))
                t = wk2.tile([128, C], BF16, tag=f"vb{pi}")
                nc.vector.tensor_copy(out=t[0:jw, :], in_=pv[0:jw, :])
                vb.append(t)

            # ---- F1: QK^T + exp + E-mult per head ----
            paA, paB = [], []
            for h in range(HEADS):
                tc4, ro = h // 4, 32 * (h % 4)
                attA = pat()
                attB = pat()
                for i0, iw in ISL:
                    nc.tensor.matmul(
                        attA[:, i0:i0 + iw], kb[tc4][ro:ro + 32, 0:128],
                        qb[tc4][ro:ro + 32, i0:i0 + iw], start=True, stop=True,
                        tile_position=(ro, 0))
                    nc.tensor.matmul(
                        attB[0:68, i0:i0 + iw], kb[tc4][ro:ro + 32, 128:NK],
                        qb[tc4][ro:ro + 32, i0:i0 + iw], start=True, stop=True,
                        tile_position=(ro, 0))
                pA = wk1.tile([128, N], BF16, tag=f"paA{h}")
                nc.scalar.activation(out=pA, in_=attA, func=AF.Exp)
                nc.vector.tensor_mul(out=pA, in0=pA, in1=Ec[h][:, 0:N])
                pB = wk1.tile([128, N], BF16, tag=f"paB{h}")
                nc.scalar.activation(out=pB[0:68, :], in_=attB[0:68, :], func=AF.Exp)
                nc.vector.tensor_mul(
                    out=pB[0:68, :], in0=pB[0:68, :], in1=Ec[h][0:68, N:2 * N])
                paA.append(pA)
                paB.append(pB)

            # ---- F2: softmax denominators, replicated per 32-row head block ----
            rS = []
            for tc4 in range(2):
                S_ps = pat()
                for i0, iw in ISL:
                    for qq in range(4):
                        h = tc4 * 4 + qq
                        nc.tensor.matmul(
                            S_ps[:, i0:i0 + iw], bh[qq][0:128, :],
                            paA[h][:, i0:i0 + iw], start=(qq == 0), stop=False)
                        nc.tensor.matmul(
                            S_ps[:, i0:i0 + iw], bh[qq][0:68, :],
                            paB[h][0:68, i0:i0 + iw], start=False,
                            stop=(qq == 3))
                r = wk2.tile([128, N], F32, tag="rS")
                nc.vector.reciprocal(out=r, in_=S_ps)
                rS.append(r)

            # ---- F3: PV -> Tun, normalize ----
            tun = [pat(), pat()]
            for h in range(HEADS):
                tc4, ro = h // 4, 32 * (h % 4)
                for i0, iw in ISL:
                    nc.tensor.matmul(
                        tun[tc4][ro:ro + 32, i0:i0 + iw],
                        vb[0][0:128, 32 * h:32 * h + 32],
                        paA[h][:, i0:i0 + iw], start=True, stop=False,
                        tile_position=(0, ro))
                    nc.tensor.matmul(
                        tun[tc4][ro:ro + 32, i0:i0 + iw],
                        vb[1][0:68, 32 * h:32 * h + 32],
                        paB[h][0:68, i0:i0 + iw], start=False, stop=True,
                        tile_position=(0, ro))
            tnb = []
            for tc4 in range(2):
                t = wk2.tile([128, N], BF16, tag=f"tnb{tc4}", bufs=1)
                nc.vector.tensor_mul(
                    out=t, in0=tun[tc4], in1=rS[tc4])
                tnb.append(t)

            # ---- F4: out-proj; SBUF->SBUF reshape-DMA does the raw
            # reinterpret ([98,256] o-chunk == rows 32j:32j+32 of [256,784]) ----
            ore = [wk2.tile([128, N], F32, tag="ore0", name="ore0", bufs=1),
                   wk2.tile([128, N], F32, tag="ore1", name="ore1", bufs=1)]
            for j in range(8):
                n0 = j * 98
                po = psm((128, C))
                nc.tensor.matmul(
                    po[0:98, :], ones1[0:1, 0:98], bo_r, start=True, stop=False)
                for tc4 in range(2):
                    nc.tensor.matmul(
                        po[0:98, :], tnb[tc4][:, n0:n0 + 98], woT[tc4],
                        start=False, stop=(tc4 == 1))
                osb = wk2.tile([128, C], F32, tag="osb")
                nc.vector.tensor_copy(out=osb[0:98, :], in_=po[0:98, :])
                nc.sync.dma_start(
                    out=scr_d[s, n0 * C:(n0 + 98) * C].rearrange(
                        "(n c) -> n c", c=C),
                    in_=osb[0:98, :])
                nc.scalar.dma_start(
                    out=ore[j // 4][32 * (j % 4):32 * (j % 4) + 32, :],
                    in_=scr_d[s, j * 25088:(j + 1) * 25088].rearrange(
                        "(a i) -> a i", i=N))

            # ---- F5: residual + LN2 ----
            x2 = []
            for ch in range(2):
                t = wk2.tile([128, N], F32, tag=f"x2{ch}", bufs=1)
                nc.vector.tensor_add(out=t, in0=ore[ch], in1=x1[ch])
                x2.append(t)
            mean2, rstd2 = ln_stats(x2, "l2")
            ln2b = []
            for ch in range(2):
                t = wk2.tile([128, N], BF16, tag=f"ln2b{ch}")
                nc.vector.tensor_scalar(
                    out=t, in0=x2[ch], scalar1=mean2, scalar2=rstd2,
                    op0=ALU.subtract, op1=ALU.mult)
                ln2b.append(t)

            emit_ffn(s, x2, ln2b)

    nc.finalize()
    _CACHE["nc"] = nc
    return nc


def kernel(**inputs):
    nc = _build()
    x = np.ascontiguousarray(inputs["x"], dtype=np.float32)
    shared = {k: np.ascontiguousarray(v, dtype=np.float32)
              for k, v in inputs.items() if k != "x"}
    in_maps = []
    for c in range(NCORES):
        m = dict(shared)
        m["x"] = np.ascontiguousarray(x[c * S:(c + 1) * S])
        in_maps.append(m)
    res = run_bass_kernel_spmd(nc, in_maps, core_ids=list(range(NCORES)))
    out = np.concatenate([res.results[c]["y"] for c in range(NCORES)], axis=0)
    return out



# revision 5
# speedup vs baseline: 1.7273x; 1.7273x over previous
import sys

sys.path.insert(0, "/opt/trn_rl_repo")

import numpy as np  # noqa: E402
import ml_dtypes  # noqa: E402

import concourse.bass as bass  # noqa: E402
import concourse.mybir as mybir  # noqa: E402
import concourse.tile as tile  # noqa: E402
from contextlib import ExitStack  # noqa: E402
from concourse import bacc  # noqa: E402
from concourse.bass_utils import run_bass_kernel_spmd  # noqa: E402

F32 = mybir.dt.float32
BF16 = mybir.dt.bfloat16
AF = mybir.ActivationFunctionType
ALU = mybir.AluOpType
AX = mybir.AxisListType
NPBF = ml_dtypes.bfloat16

S = 4  # samples per core
C, H, W = 256, 28, 28
N = H * W  # 784
NK = 196
HEADS, DK = 8, 32
CM = 1024
SCALE = DK ** -0.5
EPS = 1e-5
INV_NTOT = 1.0 / (C * N)
ISL = [(0, 512), (512, 272)]  # bank-aligned free splits of 784
NCORES = 8

# ---- bf16 const-pack column offsets ----
O_DGLPU = 0                 # 18 * 128
O_DGKV = O_DGLPU + 18 * 128   # 8 * 128
O_DGDW2 = O_DGKV + 8 * 128    # 72 * 128
O_WQ = O_DGDW2 + 72 * 128     # 2 * 256
O_WK = O_WQ + 512
O_WV = O_WK + 512
O_WO = O_WV + 512
O_C1 = O_WO + 512             # 2 * 1024
O_C2 = O_C1 + 2048            # 8 * 256
O_BH = O_C2 + 2048            # 4 * 128
O_ONE = O_BH + 512            # 128 (ones)
O_BOR = O_ONE + 128           # 256 (row0 only)
O_BVR = O_BOR + 256           # 256 (row0 only)
WCOLS = O_BVR + 256

# ---- f32 const-pack column offsets ----
P_ONEM = 0                  # 128 (ones, f32)
P_EPS = P_ONEM + 128        # 1
P_A1 = P_EPS + 1            # 8
P_B1 = P_A1 + 8
P_A2 = P_B1 + 8
P_B2 = P_A2 + 8
P_A3 = P_B2 + 8             # 2
P_B3 = P_A3 + 2
P_LPUB = P_B3 + 2           # 2
P_DWB = P_LPUB + 2
P_BQ = P_DWB + 2
P_BK = P_BQ + 2
FCOLS = P_BK + 2

_CACHE = {}


def _prep(inputs):
    """Host-side precompute: all weight-derived constants in SBUF-ready
    layouts so the device program only DMAs them in."""
    f32 = np.float32
    wpk = np.zeros((128, WCOLS), dtype=NPBF)
    fpk = np.zeros((128, FCOLS), dtype=f32)

    def put_diags(off, w2d, G, T):
        # w2d [G*128, T]; slice (g*T+t) is diag(w2d[g*128:(g+1)*128, t])
        for g in range(G):
            for t in range(T):
                d = np.zeros((128, 128), f32)
                np.fill_diagonal(d, w2d[g * 128:(g + 1) * 128, t])
                k = off + (g * T + t) * 128
                wpk[:, k:k + 128] = d.astype(NPBF)

    put_diags(O_DGLPU, np.asarray(inputs["lpu_w"], f32).reshape(C, 9), 2, 9)
    put_diags(O_DGKV, np.asarray(inputs["dw_w"], f32).reshape(C, 4), 2, 4)
    put_diags(O_DGDW2, np.asarray(inputs["dw2_w"], f32).reshape(CM, 9), 8, 9)

    def put_wT(off, w, km, scale=1.0):
        # w [M, K] -> km tiles [128, M] ; tile kc = w[:, kc*128:+128].T
        wT = (np.asarray(w, f32).T * scale).astype(NPBF)  # [K, M]
        M = wT.shape[1]
        for kc in range(km):
            wpk[:, off + kc * M:off + (kc + 1) * M] = wT[kc * 128:(kc + 1) * 128]

    put_wT(O_WQ, inputs["wq"], 2, SCALE)
    put_wT(O_WK, inputs["wk"], 2)
    put_wT(O_WV, inputs["wv"], 2)
    put_wT(O_WO, inputs["wo"], 2)
    put_wT(O_C1, np.asarray(inputs["c1_w"], f32).reshape(CM, C), 2)
    put_wT(O_C2, np.asarray(inputs["c2_w"], f32).reshape(C, CM), 8)

    for q in range(4):
        wpk[:, O_BH + q * 128 + 32 * q:O_BH + q * 128 + 32 * q + 32] = NPBF(1.0)
    wpk[:, O_ONE:O_ONE + 128] = NPBF(1.0)
    wpk[0, O_BOR:O_BOR + 256] = np.asarray(inputs["bo"], f32).astype(NPBF)
    wpk[0, O_BVR:O_BVR + 256] = np.asarray(inputs["bv"], f32).astype(NPBF)

    # exp(pos_b)^T per head, packed [128, 1568] each
    pos = np.asarray(inputs["pos_b"], f32)[0]          # [8, 784, 196]
    e = np.exp(pos.transpose(0, 2, 1))                 # [8, 196, 784]
    et = np.zeros((128, HEADS * 2 * N), dtype=NPBF)
    for h in range(HEADS):
        et[:, h * 2 * N:h * 2 * N + N] = e[h, :128].astype(NPBF)
        et[:68, h * 2 * N + N:(h + 1) * 2 * N] = e[h, 128:].astype(NPBF)

    fpk[:, P_ONEM:P_ONEM + 128] = 1.0
    fpk[:, P_EPS] = EPS

    def bn_fold(offA, offB, g, b, m, v, cb, G):
        g, b, m, v, cb = (np.asarray(t, f32) for t in (g, b, m, v, cb))
        A = g / np.sqrt(v + EPS)
        B = b - m * A + A * cb
        fpk[:, offA:offA + G] = A.reshape(G, 128).T
        fpk[:, offB:offB + G] = B.reshape(G, 128).T

    bn_fold(P_A1, P_B1, inputs["bn1_g"], inputs["bn1_b"], inputs["bn1_m"],
            inputs["bn1_v"], inputs["c1_b"], 8)
    bn_fold(P_A2, P_B2, inputs["bn2_g"], inputs["bn2_b"], inputs["bn2_m"],
            inputs["bn2_v"], inputs["dw2_b"], 8)
    bn_fold(P_A3, P_B3, inputs["bn3_g"], inputs["bn3_b"], inputs["bn3_m"],
            inputs["bn3_v"], inputs["c2_b"], 2)

    fpk[:, P_LPUB:P_LPUB + 2] = np.asarray(inputs["lpu_b"], f32).reshape(2, 128).T
    fpk[:, P_DWB:P_DWB + 2] = np.asarray(inputs["dw_b"], f32).reshape(2, 128).T
    fpk[:, P_BQ:P_BQ + 2] = (np.asarray(inputs["bq"], f32) * SCALE).reshape(2, 128).T
    fpk[:, P_BK:P_BK + 2] = np.asarray(inputs["bk"], f32).reshape(2, 128).T

    return wpk, fpk, et


def _build():
    if "nc" in _CACHE:
        return _CACHE["nc"]
    nc = bacc.Bacc()

    x_d = nc.dram_tensor("x", [S, C, H, W], F32, kind="ExternalInput")
    y_d = nc.dram_tensor("y", [S, C, H, W], F32, kind="ExternalOutput")
    scr_d = nc.dram_tensor("scr", [S, N * C], F32)
    wpk_d = nc.dram_tensor("wpk", [128, WCOLS], BF16, kind="ExternalInput")
    fpk_d = nc.dram_tensor("fpk", [128, FCOLS], F32, kind="ExternalInput")
    et_d = nc.dram_tensor("et", [128, HEADS * 2 * N], BF16, kind="ExternalInput")

    xv = x_d.rearrange("s c h w -> s c (h w)")
    yv = y_d.rearrange("s c h w -> s c (h w)")

    with tile.TileContext(nc) as tc, ExitStack() as stk:
        cst = stk.enter_context(tc.tile_pool(name="cst", bufs=1))
        wk2 = stk.enter_context(tc.tile_pool(name="wk2", bufs=2))
        wk1 = stk.enter_context(tc.tile_pool(name="wk1", bufs=1))
        psA = stk.enter_context(tc.tile_pool(name="psA", bufs=3, space="PSUM"))
        psS = stk.enter_context(tc.tile_pool(name="psS", bufs=2, space="PSUM"))

        def pat(shape=(128, N)):
            return psA.tile(list(shape), F32, tag="attn", name="pat")

        def psm(shape=(128, 392)):
            return psS.tile(list(shape), F32, tag="small", name="psm")

        # ---------- resident constants (pure DMA) ----------
        wpk = cst.tile([128, WCOLS], BF16, tag="wpk")
        nc.scalar.dma_start(out=wpk, in_=wpk_d[:, :])
        fpk = cst.tile([128, FCOLS], F32, tag="fpk")
        nc.scalar.dma_start(out=fpk, in_=fpk_d[:, :])
        et = cst.tile([128, HEADS * 2 * N], BF16, tag="et")
        nc.gpsimd.dma_start(out=et, in_=et_d[:, :])

        def wcol(off, w):
            return wpk[:, off:off + w]

        dg_lpu = [[wcol(O_DGLPU + (g * 9 + t) * 128, 128) for t in range(9)]
                  for g in range(2)]
        dg_kv = [[wcol(O_DGKV + (g * 4 + t) * 128, 128) for t in range(4)]
                 for g in range(2)]
        dg_dw2 = [[wcol(O_DGDW2 + (g * 9 + t) * 128, 128) for t in range(9)]
                  for g in range(8)]
        wqT = [wcol(O_WQ + kc * 256, 256) for kc in range(2)]
        wkT = [wcol(O_WK + kc * 256, 256) for kc in range(2)]
        wvT = [wcol(O_WV + kc * 256, 256) for kc in range(2)]
        woT = [wcol(O_WO + kc * 256, 256) for kc in range(2)]
        c1T = [wcol(O_C1 + kc * 1024, 1024) for kc in range(2)]
        c2T = [wcol(O_C2 + kc * 256, 256) for kc in range(8)]
        bh = [wcol(O_BH + q * 128, 128) for q in range(4)]
        ones1 = wpk[0:1, O_ONE:O_ONE + 128]
        bo_r = wpk[0:1, O_BOR:O_BOR + 256]
        bv_r = wpk[0:1, O_BVR:O_BVR + 256]
        Ec = [et[:, h * 2 * N:(h + 1) * 2 * N] for h in range(HEADS)]

        onesM = fpk[:, P_ONEM:P_ONEM + 128]
        eps128 = fpk[:, P_EPS:P_EPS + 1]
        A1 = fpk[:, P_A1:P_A1 + 8]; B1 = fpk[:, P_B1:P_B1 + 8]
        A2 = fpk[:, P_A2:P_A2 + 8]; B2 = fpk[:, P_B2:P_B2 + 8]
        A3 = fpk[:, P_A3:P_A3 + 2]; B3 = fpk[:, P_B3:P_B3 + 2]
        lpub = [fpk[:, P_LPUB + g:P_LPUB + g + 1] for g in range(2)]
        dwb = [fpk[:, P_DWB + g:P_DWB + g + 1] for g in range(2)]
        bqc = [fpk[:, P_BQ + g:P_BQ + g + 1] for g in range(2)]
        bkc = [fpk[:, P_BK + g:P_BK + g + 1] for g in range(2)]

        # LN over (C,H,W): returns (mean, rstd) as [128,1] APs
        def ln_stats(chunks, tagp):
            st4 = wk2.tile([128, 4], F32, tag=f"st4{tagp}")
            for ch in range(2):
                nc.vector.tensor_reduce(
                    out=st4[:, 2 * ch:2 * ch + 1], in_=chunks[ch], axis=AX.X,
                    op=ALU.add)
                scr = wk2.tile([128, N], BF16, tag="lnsc")
                nc.scalar.activation(
                    out=scr, in_=chunks[ch], func=AF.Square,
                    accum_out=st4[:, 2 * ch + 1:2 * ch + 2])
            pst = psm((128, 4))
            nc.tensor.matmul(pst, onesM, st4, start=True, stop=True)
            stc = wk2.tile([128, 4], F32, tag=f"stc{tagp}")
            nc.vector.tensor_copy(out=stc, in_=pst)
            tot = wk2.tile([128, 2], F32, tag=f"tot{tagp}")
            nc.vector.tensor_add(out=tot, in0=stc[:, 0:2], in1=stc[:, 2:4])
            mv = wk2.tile([128, 2], F32, tag=f"mv{tagp}")
            nc.vector.tensor_scalar(
                out=mv, in0=tot, scalar1=INV_NTOT, scalar2=None, op0=ALU.mult)
            m2 = wk2.tile([128, 1], F32, tag=f"m2{tagp}")
            nc.vector.tensor_mul(out=m2, in0=mv[:, 0:1], in1=mv[:, 0:1])
            var = wk2.tile([128, 1], F32, tag=f"var{tagp}")
            nc.vector.tensor_sub(out=var, in0=mv[:, 1:2], in1=m2)
            lg = wk2.tile([128, 1], F32, tag=f"lg{tagp}")
            nc.scalar.activation(out=lg, in_=var, func=AF.Ln, bias=eps128)
            rst = wk2.tile([128, 1], F32, tag=f"rst{tagp}")
            nc.scalar.activation(out=rst, in_=lg, func=AF.Exp, scale=-0.5)
            return mv[:, 0:1], rst

        def emit_ffn(s, x2, ln2b):
            h1p = []
            for mc in range(8):
                pc1 = pat()
                for i0, iw in ISL:
                    for kc in range(2):
                        nc.tensor.matmul(
                            pc1[:, i0:i0 + iw],
                            c1T[kc][:, mc * 128:(mc + 1) * 128],
                            ln2b[kc][:, i0:i0 + iw],
                            start=(kc == 0), stop=(kc == 1))
                hp = wk1.tile([128, 30, 30], BF16, tag=f"h1p{mc}", name="hp")
                if s == 0:
                    nc.vector.memset(hp, 0.0)
                nc.scalar.activation(
                    out=hp[:, 1:29, 1:29],
                    in_=pc1.rearrange("p (h w) -> p h w", w=W),
                    func=AF.Gelu, scale=A1[:, mc:mc + 1], bias=B1[:, mc:mc + 1])
                h1p.append(hp)
            h2 = []
            for mc in range(8):
                t = wk1.tile([128, N], BF16, tag=f"h2{mc}", name="t")
                for hf in range(2):
                    pd = psm()
                    for t9 in range(9):
                        dy, dx = t9 // 3, t9 % 3
                        nc.tensor.matmul(
                            pd, dg_dw2[mc][t9],
                            h1p[mc][:, dy + 14 * hf:dy + 14 * hf + 14, dx:dx + 28],
                            start=(t9 == 0), stop=(t9 == 8))
                    nc.scalar.activation(
                        out=t[:, hf * 392:(hf + 1) * 392], in_=pd,
                        func=AF.Gelu, scale=A2[:, mc:mc + 1], bias=B2[:, mc:mc + 1])
                h2.append(t)
            for mc in range(2):
                pc2 = pat()
                for i0, iw in ISL:
                    for kc in range(8):
                        nc.tensor.matmul(
                            pc2[:, i0:i0 + iw],
                            c2T[kc][:, mc * 128:(mc + 1) * 128],
                            h2[kc][:, i0:i0 + iw],
                            start=(kc == 0), stop=(kc == 7))
                t3 = wk2.tile([128, N], F32, tag="t3")
                nc.vector.tensor_scalar(
                    out=t3, in0=pc2, scalar1=A3[:, mc:mc + 1],
                    scalar2=B3[:, mc:mc + 1], op0=ALU.mult, op1=ALU.add)
                nc.vector.tensor_add(out=t3, in0=t3, in1=x2[mc])
                nc.sync.dma_start(
                    out=yv[s, mc * 128:(mc + 1) * 128, :], in_=t3)

        # ================= per-sample pipeline =================
        for s in range(S):
            # ---- A: load + LPU ----
            xs, xb, x1, x1b = [], [], [], []
            for ch in range(2):
                t = wk2.tile([128, N], F32, tag="xs", name="t")
                nc.sync.dma_start(out=t, in_=xv[s, ch * 128:(ch + 1) * 128, :])
                xs.append(t)
                p = wk2.tile([128, 30, 30], BF16, tag=f"xb{ch}", bufs=1, name="p")
                if s == 0:
                    nc.vector.memset(p, 0.0)
                nc.gpsimd.tensor_copy(
                    out=p[:, 1:29, 1:29],
                    in_=t.rearrange("p (h w) -> p h w", w=W))
                xb.append(p)
            for ch in range(2):
                t = wk2.tile([128, N], F32, tag=f"x1{ch}", bufs=1, name="t")
                for hf in range(2):
                    pl = psm()
                    for t9 in range(9):
                        dy, dx = t9 // 3, t9 % 3
                        nc.tensor.matmul(
                            pl, dg_lpu[ch][t9],
                            xb[ch][:, dy + 14 * hf:dy + 14 * hf + 14, dx:dx + 28],
                            start=(t9 == 0), stop=(t9 == 8))
                    nc.vector.scalar_tensor_tensor(
                        out=t[:, hf * 392:(hf + 1) * 392], in0=pl,
                        scalar=lpub[ch], in1=xs[ch][:, hf * 392:(hf + 1) * 392],
                        op0=ALU.add, op1=ALU.add)
                x1.append(t)
                tb = wk2.tile([128, N], BF16, tag=f"x1b{ch}", name="tb")
                nc.gpsimd.tensor_copy(out=tb, in_=t)
                x1b.append(tb)

            # ---- C: kv conv (2x2 stride 2 on x1) ----
            kvb = []
            for ch in range(2):
                x5 = x1b[ch].rearrange(
                    "p (h a w b) -> p h a w b", h=14, a=2, w=14, b=2)
                pk = psm((128, NK))
                for t4 in range(4):
                    dy, dx = t4 // 2, t4 % 2
                    nc.tensor.matmul(
                        pk, dg_kv[ch][t4], x5[:, :, dy, :, dx],
                        start=(t4 == 0), stop=(t4 == 3))
                t = wk2.tile([128, NK], BF16, tag=f"kvb{ch}", name="t")
                nc.scalar.activation(out=t, in_=pk, func=AF.Identity, bias=dwb[ch])
                kvb.append(t)

            # ---- B: LN1 ----
            mean1, rstd1 = ln_stats(x1, "l1")
            ln1b = []
            for ch in range(2):
                t = wk2.tile([128, N], BF16, tag=f"ln1b{ch}", name="t")
                nc.vector.tensor_scalar(
                    out=t, in0=x1[ch], scalar1=mean1, scalar2=rstd1,
                    op0=ALU.subtract, op1=ALU.mult)
                ln1b.append(t)

            # ---- D: k/v then q projections ----
            kb = []
            for mc in range(2):
                pk2 = psm((128, NK))
                for kc in range(2):
                    nc.tensor.matmul(
                        pk2, wkT[kc][:, mc * 128:(mc + 1) * 128], kvb[kc],
                        start=(kc == 0), stop=(kc == 1))
                t = wk2.tile([128, NK], BF16, tag=f"kb{mc}", name="t")
                nc.scalar.activation(out=t, in_=pk2, func=AF.Identity, bias=bkc[mc])
                kb.append(t)
            vb = []
            for pi, (j0, jw) in enumerate([(0, 128), (128, 68)]):
                pv = psm((128, C))
                nc.tensor.matmul(
                    pv[0:jw, :], ones1[0:1, 0:jw], bv_r, start=True, stop=False)
                for kc in range(2):
                    nc.tensor.matmul(
                        pv[0:jw, :], kvb[kc][:, j0:j0 + jw], wvT[kc],
                        start=False, stop=(kc == 1))
                t = wk2.tile([128, C], BF16, tag=f"vb{pi}", name="t")
                nc.scalar.activation(out=t[0:jw, :], in_=pv[0:jw, :],
                                     func=AF.Identity)
                vb.append(t)
            qb = []
            for mc in range(2):
                pq = pat()
                for i0, iw in ISL:
                    for kc in range(2):
                        nc.tensor.matmul(
                            pq[:, i0:i0 + iw],
                            wqT[kc][:, mc * 128:(mc + 1) * 128],
                            ln1b[kc][:, i0:i0 + iw],
                            start=(kc == 0), stop=(kc == 1))
                t = wk2.tile([128, N], BF16, tag=f"qb{mc}", name="t")
                nc.scalar.activation(out=t, in_=pq, func=AF.Identity,
                                     bias=bqc[mc])
                qb.append(t)

            # ---- F1: QK^T + exp + E-mult per head ----
            paA, paB = [], []
            for h in range(HEADS):
                tc4, ro = h // 4, 32 * (h % 4)
                attA = pat()
                attB = pat()
                for i0, iw in ISL:
                    nc.tensor.matmul(
                        attA[:, i0:i0 + iw], kb[tc4][ro:ro + 32, 0:128],
                        qb[tc4][ro:ro + 32, i0:i0 + iw], start=True, stop=True,
                        tile_position=(ro, 0))
                    nc.tensor.matmul(
                        attB[0:68, i0:i0 + iw], kb[tc4][ro:ro + 32, 128:NK],
                        qb[tc4][ro:ro + 32, i0:i0 + iw], start=True, stop=True,
                        tile_position=(ro, 0))
                pA = wk1.tile([128, N], BF16, tag=f"paA{h}", name="pA")
                nc.scalar.activation(out=pA, in_=attA, func=AF.Exp)
                nc.vector.tensor_mul(out=pA, in0=pA, in1=Ec[h][:, 0:N])
                pB = wk1.tile([128, N], BF16, tag=f"paB{h}", name="pB")
                nc.scalar.activation(out=pB[0:68, :], in_=attB[0:68, :], func=AF.Exp)
                nc.vector.tensor_mul(
                    out=pB[0:68, :], in0=pB[0:68, :], in1=Ec[h][0:68, N:2 * N])
                paA.append(pA)
                paB.append(pB)

            # ---- F2: softmax denominators, replicated per 32-row head block ----
            rS = []
            for tc4 in range(2):
                S_ps = pat()
                for i0, iw in ISL:
                    for qq in range(4):
                        h = tc4 * 4 + qq
                        nc.tensor.matmul(
                            S_ps[:, i0:i0 + iw], bh[qq][0:128, :],
                            paA[h][:, i0:i0 + iw], start=(qq == 0), stop=False)
                        nc.tensor.matmul(
                            S_ps[:, i0:i0 + iw], bh[qq][0:68, :],
                            paB[h][0:68, i0:i0 + iw], start=False,
                            stop=(qq == 3))
                r = wk2.tile([128, N], F32, tag="rS", name="r")
                nc.vector.reciprocal_approx_fast(out=r, in_=S_ps)
                rS.append(r)

            # ---- F3: PV -> Tun, normalize ----
            tun = [pat(), pat()]
            tnb = [None, None]
            for h in range(HEADS):
                tc4, ro = h // 4, 32 * (h % 4)
                for i0, iw in ISL:
                    nc.tensor.matmul(
                        tun[tc4][ro:ro + 32, i0:i0 + iw],
                        vb[0][0:128, 32 * h:32 * h + 32],
                        paA[h][:, i0:i0 + iw], start=True, stop=False,
                        tile_position=(0, ro))
                    nc.tensor.matmul(
                        tun[tc4][ro:ro + 32, i0:i0 + iw],
                        vb[1][0:68, 32 * h:32 * h + 32],
                        paB[h][0:68, i0:i0 + iw], start=False, stop=True,
                        tile_position=(0, ro))
                if h % 4 == 3:
                    t = wk2.tile([128, N], BF16, tag=f"tnb{tc4}", bufs=1,
                                 name="t")
                    nc.vector.tensor_mul(out=t, in0=tun[tc4], in1=rS[tc4])
                    tnb[tc4] = t

            # ---- F4: out-proj; SBUF->SBUF reshape-DMA does the raw
            # reinterpret ([98,256] o-chunk == rows 32j:32j+32 of [256,784]) ----
            ore = [wk2.tile([128, N], F32, tag="ore0", name="ore0", bufs=1),
                   wk2.tile([128, N], F32, tag="ore1", name="ore1", bufs=1)]
            for j in range(8):
                n0 = j * 98
                po = psm((128, C))
                nc.tensor.matmul(
                    po[0:98, :], ones1[0:1, 0:98], bo_r, start=True, stop=False)
                for tc4 in range(2):
                    nc.tensor.matmul(
                        po[0:98, :], tnb[tc4][:, n0:n0 + 98], woT[tc4],
                        start=False, stop=(tc4 == 1))
                osb = wk2.tile([128, C], F32, tag="osb", name="osb")
                nc.scalar.activation(out=osb[0:98, :], in_=po[0:98, :],
                                     func=AF.Identity)
                nc.sync.dma_start(
                    out=scr_d[s, n0 * C:(n0 + 98) * C].rearrange(
                        "(n c) -> n c", c=C),
                    in_=osb[0:98, :])
                nc.scalar.dma_start(
                    out=ore[j // 4][32 * (j % 4):32 * (j % 4) + 32, :],
                    in_=scr_d[s, j * 25088:(j + 1) * 25088].rearrange(
                        "(a i) -> a i", i=N))

            # ---- F5: residual + LN2 ----
            x2 = []
            for ch in range(2):
                t = wk2.tile([128, N], F32, tag=f"x2{ch}", bufs=1, name="t")
                nc.vector.tensor_add(out=t, in0=ore[ch], in1=x1[ch])
                x2.append(t)
            mean2, rstd2 = ln_stats(x2, "l2")
            ln2b = []
            for ch in range(2):
                t = wk2.tile([128, N], BF16, tag=f"ln2b{ch}", name="t")
                nc.vector.tensor_scalar(
                    out=t, in0=x2[ch], scalar1=mean2, scalar2=rstd2,
                    op0=ALU.subtract, op1=ALU.mult)
                ln2b.append(t)

            emit_ffn(s, x2, ln2b)

    nc.finalize()
    _CACHE["nc"] = nc
    return nc


def _in_maps(inputs):
    wpk, fpk, et = _prep(inputs)
    x = np.ascontiguousarray(inputs["x"], dtype=np.float32)
    in_maps = []
    for c in range(NCORES):
        in_maps.append(dict(
            x=np.ascontiguousarray(x[c * S:(c + 1) * S]),
            wpk=wpk, fpk=fpk, et=et))
    return in_maps


def kernel(**inputs):
    nc = _build()
    res = run_bass_kernel_spmd(nc, _in_maps(inputs), core_ids=list(range(NCORES)))
    out = np.concatenate([res.results[c]["y"] for c in range(NCORES)], axis=0)
    return out
